# revision 52
# baseline (speedup 1.0000x reference)
"""ASTRF kernel for Trainium2 (8 NeuronCores, axon).

Math: out[b,o,t] = sum_{i,w} weight[o,i,w] * xs[b,i,t-w] + bias[o]
where xs[b,i,src[b,s]] = x[b,i,s] (scatter of events to onsets).

Device scheme (per core = 2 jobs of (batch, 128-block group)):
  time t = 64*m + q, q = 8c + q'.  Out[m, tile c] =
      sum_{(u,i)} Xblk[m][(u,i)] * W0[c-a] + Xblk[m-1][(u,i)] * W1[a-c]
  K = (u,i) = 1024 -> 8 chunks of 128 partitions; M = 128 blocks; N = 512
  (n = q'*64 + o).  Weights live in one shifted tensor
  Wsh[(u',i), zz', o] = weight[o,i,zz'-u'] (zero-padded): tile c's 9
  accumulating matmuls read ascending 8-wide windows zz' in [8k, 8k+8).
  Banks are processed in PAIRS (one [128,1024] 2-bank PSUM tile, matmuls
  window-interleaved), drained by tensor_copy alternating DVE/ACT into a
  staging tile, then one contiguous 512KB DMA per pair.  Bias is added on
  the host during unshard (trivial broadcast add).
"""

import sys

for _p in ("/opt/trn_rl_repo", "/root/.axon_site/_ro/trn_rl_repo"):
    if _p not in sys.path:
        sys.path.insert(0, _p)

import numpy as np

B, I, S = 4, 16, 4096
O, W = 64, 64
T = 32768
NBLK = T // 64            # 512 blocks per batch
N_CORES = 8

_prog_cache = {}


def _mats_for(c):
    """(chunk a, wsh window k, lhsT col offset) in ascending-k (DMA) order."""
    mats = [(c, 0, 1)]                                   # W0[0]
    mats += [(c - k, k, 1) for k in range(1, c + 1)]     # W0[k]
    mats += [(c + 8 - k, k, 0) for k in range(c + 1, 9)]  # W1[8-k]
    return mats


def _build_program():
    if "nc" in _prog_cache:
        return _prog_cache["nc"]
    import concourse.bacc as bacc
    import concourse.mybir as mybir
    import concourse.tile as tile

    f32 = mybir.dt.float32
    f32r = mybir.dt.float32r
    nc = bacc.Bacc("TRN2", target_bir_lowering=False, debug=False, num_devices=N_CORES)

    xin = nc.dram_tensor("xin", [128, 2, 8, 129], f32r, kind="ExternalInput")
    wshd = nc.dram_tensor("wshd", [128, 72, 64], f32r, kind="ExternalInput")
    out = nc.dram_tensor("out", [2, 4, 128, 1024], f32, kind="ExternalOutput")

    with tile.TileContext(nc) as tc:
        with (
            tc.tile_pool(name="const", bufs=1) as cpool,
            tc.tile_pool(name="stage", bufs=3) as spool,
            tc.tile_pool(name="psum", bufs=1, space="PSUM") as ppool,
        ):
            x_sb = cpool.tile([128, 2, 8, 129], f32r, tag="x")
            wsh_sb = cpool.tile([128, 72, 64], f32r, tag="wsh")
            scr = cpool.tile([128, 512], f32, tag="scr")
            wps = ppool.tile([128, 512], f32, tag="ps0", name="warm_ps")
            nc.gpsimd.memset(scr[:], 0.0)
            for _w in range(7):
                nc.tensor.matmul(
                    out=wps[:, 0:128],
                    lhsT=scr[:, 0:128],
                    rhs=scr[:, 0:128],
                    start=(_w == 0),
                    stop=(_w == 6),
                )
            # input DMAs issued from ACT, outputs from SP
            nc.scalar.dma_start(
                out=wsh_sb[:, 0:8, :], in_=wshd[:, 0:8, :]
            )
            nc.scalar.dma_start(out=x_sb[:, 0, 0:4], in_=xin[:, 0, 0:4])
            nc.scalar.dma_start(out=x_sb[:, 0, 4:8], in_=xin[:, 0, 4:8])
            for k in range(1, 9):
                nc.scalar.dma_start(
                    out=wsh_sb[:, 8 * k : 8 * k + 8, :],
                    in_=wshd[:, 8 * k : 8 * k + 8, :],
                )
            nc.scalar.dma_start(out=x_sb[:, 1], in_=xin[:, 1])

            def mat(j, pslc, a, kk, col0, start, stop):
                nc.tensor.matmul(
                    out=pslc,
                    lhsT=x_sb[:, j, a, col0 : col0 + 128],
                    rhs=wsh_sb[:, 8 * kk : 8 * kk + 8, :],
                    start=start,
                    stop=stop,
                )

            def drain(j, p, ps):
                stage = spool.tile(
                    [128, 1024], f32, tag="stage", name=f"st{j}_{p}"
                )
                if p % 2 == 0:
                    nc.vector.tensor_copy(out=stage[:], in_=ps[:])
                else:
                    nc.scalar.copy(out=stage[:], in_=ps[:])
                nc.sync.dma_start(out=out[j, p], in_=stage[:])

            def drain3(j, psA, psB):
                # non-tail pair-3 drain from the two 1-bank tiles
                stage = spool.tile(
                    [128, 1024], f32, tag="stage", name=f"st{j}_3"
                )
                nc.vector.tensor_copy(out=stage[:, 0:512], in_=psA[:])
                nc.scalar.copy(out=stage[:, 512:1024], in_=psB[:])
                nc.sync.dma_start(out=out[j, 3], in_=stage[:])

            def drain_half3(j, pslc, half):
                # pipelined half drains for the tail pair
                stage = spool.tile(
                    [128, 1024], f32, tag="stage", name=f"st{j}_3"
                ) if half == 1 else drain_half3.stage
                drain_half3.stage = stage
                sl = slice(512 * half, 512 * half + 512)
                if half == 1:
                    nc.vector.tensor_copy(out=stage[:, sl], in_=pslc)
                else:
                    nc.scalar.copy(out=stage[:, sl], in_=pslc)
                nc.sync.dma_start(out=out[j, 3, :, sl], in_=stage[:, sl])

            # --- both jobs pair-major, consecutive matmuls share lhsT
            # (m0[k] and m1[k+1] use the same chunk+offset -> one weight
            # load feeds two matmuls on HW); split-engine drain on tail ---
            mats_of = {c: _mats_for(c) for c in range(8)}
            for j in range(2):
                for p in range(4):
                    if p < 3:
                        ps = ppool.tile(
                            [128, 1024], f32, tag=f"ps{p}", name=f"ps{j}_{p}"
                        )
                        slc = {0: ps[:, 0:512], 1: ps[:, 512:1024]}
                    else:
                        # pair 3 uses two separate 1-bank tiles so the tail
                        # half-drain never serializes against the PE writes
                        # of the other half (same-tile PSUM hazard)
                        psA = ppool.tile(
                            [128, 512], f32, tag="ps3a", name=f"ps{j}_3a"
                        )
                        psB = ppool.tile(
                            [128, 512], f32, tag="ps3b", name=f"ps{j}_3b"
                        )
                        slc = {0: psA[:], 1: psB[:]}
                    m0 = mats_of[2 * p]
                    m1 = mats_of[2 * p + 1]
                    tail = j == 1 and p == 3
                    if not tail:
                        seq = [(1, m1[0])]
                        for k in range(8):
                            seq += [(0, m0[k]), (1, m1[k + 1])]
                        seq += [(0, m0[8])]
                    else:
                        # finish half1 early so its drain+DMA overlap the
                        # last half0 matmuls
                        seq = [(1, m1[0])]
                        for k in range(5):
                            seq += [(0, m0[k]), (1, m1[k + 1])]
                        seq += [(1, m1[6]), (1, m1[7]), (1, m1[8])]
                        seq += [(0, m0[5]), (0, m0[6]), (0, m0[7]), (0, m0[8])]
                    nseen = {0: 0, 1: 0}
                    for half, (a, kk, col0) in seq:
                        nseen[half] += 1
                        mat(j, slc[half], a, kk, col0,
                            nseen[half] == 1, nseen[half] == 9)
                        if tail and nseen[half] == 9:
                            drain_half3(j, slc[half], half)
                    if not tail:
                        if p < 3:
                            drain(j, p, ps)
                        else:
                            drain3(j, psA, psB)

    nc.compile()
    _prog_cache["nc"] = nc
    return nc


def _host_pack(x, weight, sourceIdx):
    """Build per-core device inputs from full inputs."""
    # --- scatter x into blocked layout xs6[b, a, u', i, col=m+1] ---
    xs6 = np.zeros((B, 8, 8, I, NBLK + 1), np.float32)
    src = np.asarray(sourceIdx, np.int64)
    for b in range(B):
        t = src[b]
        m = t >> 6
        u = t & 63
        xs6[b, u >> 3, u & 7, :, m + 1] = np.asarray(x[b], np.float32).T
    # per-core X: [p=u'*16+i, j, a, col']
    x_cores = []
    for core in range(N_CORES):
        b, h = divmod(core, 2)
        tmp = xs6[b].transpose(1, 2, 0, 3).reshape(128, 8, NBLK + 1)
        arr = np.empty((128, 2, 8, 129), np.float32)
        for j in range(2):
            g = 2 * h + j
            arr[:, j, :, :] = tmp[:, :, 128 * g : 128 * g + 129]
        x_cores.append(np.ascontiguousarray(arr))

    # --- shifted weights Wsh[p=(u',i), zz', o] = weight[o,i,zz'-u'] ---
    wgt = np.asarray(weight, np.float32)  # (O, I, W)
    zz = np.arange(72)
    up = np.arange(8)
    idx = zz[None, :] - up[:, None]              # (8 u', 72 zz')
    valid = (idx >= 0) & (idx < W)
    g = wgt[:, :, np.clip(idx, 0, W - 1)] * valid[None, None]  # (O, I, 8, 72)
    wsh_host = np.ascontiguousarray(
        g.transpose(2, 1, 3, 0).reshape(128, 72, 64), dtype=np.float32
    )
    return x_cores, wsh_host


def kernel(x, weight, bias, sourceIdx, nRealLen, _trace=False, _trace_out=None):
    import jax

    from concourse import bass_utils

    # the device run needs the 8 axon NeuronCores; if the caller pinned jax
    # to another platform (e.g. cpu for a reference computation), switch back
    if len(jax.devices()) < N_CORES:
        jax.config.update("jax_platforms", "axon")
        try:
            import jax.extend.backend

            jax.extend.backend.clear_backends()
        except Exception:
            pass
        assert len(jax.devices()) >= N_CORES, (
            f"need {N_CORES} neuron cores, have {jax.devices()}"
        )

    nRealLen = int(nRealLen)
    assert nRealLen == T, f"kernel hardcoded for nRealLen={T}, got {nRealLen}"
    x_cores, wsh_host = _host_pack(x, weight, sourceIdx)
    nc = _build_program()
    in_maps = [{"xin": x_cores[c], "wshd": wsh_host} for c in range(N_CORES)]
    res = bass_utils.run_bass_kernel_spmd(
        nc,
        in_maps,
        core_ids=list(range(N_CORES)),
        trace=_trace,
        trace_cores=list(range(N_CORES)) if _trace else None,
    )
    if _trace_out is not None:
        _trace_out.append(res)
    bias_f = np.asarray(bias, np.float32)
    out_full = np.empty((B, O, T), np.float32)
    for core in range(N_CORES):
        b, h = divmod(core, 2)
        r = res.results[core]["out"]  # (2, 4, 128, 1024)
        r6 = r.reshape(2, 4, 128, 2, 8, 64)  # [j, p, m, half, q', o]
        for j in range(2):
            g = 2 * h + j
            seg = r6[j].transpose(4, 1, 0, 2, 3).reshape(64, 8192)
            out_full[b, :, g * 8192 : (g + 1) * 8192] = seg
    out_full += bias_f[None, :, None]
    return out_full
